# revision 9
# baseline (speedup 1.0000x reference)
"""Trainium2 Bass kernel for nn_Aggregator (GNN message passing + GCNII layer).

Computes, for N=100000 nodes / E=1600000 edges / D=128:
    side = segment_sum(vals * ego[col], row)          # sparse A @ ego
    hi   = ego + side
    res  = 0.9*hi + 0.1*(h0 @ w_h0.T + b_h0)
    emb  = leaky_relu(res @ IM @ w_lin.T + b_lin)     # IM = (1-b) + b*weight
    out  = layernorm(emb) * gamma + beta

Sharding: 8 cores, each owns a contiguous 12500-row slice of the output
nodes (padded to NB*128). The full ego table is replicated per core and
used as the gather source (no collectives). Edges are bucketed on the host
by (dest block of 128 rows, table quarter), padded to 128-edge groups with
a uniform per-cell group count C so all 8 cores run one identical program.

Device inner loop per 128-row block:
  dma_gather pulls each group's 128 neighbor rows onto partitions;
  DVE builds a one-hot selector S[e,n] = (iota==slot[e]) * val[e];
  TensorE accumulates side[n,f] += S.T @ G in PSUM over the block's groups;
  dense epilogue runs feature-major via PE transposes, with biases folded
  into rank-1 accumulating matmuls and LayerNorm via bn_stats/bn_aggr.
"""

import math
from contextlib import ExitStack

import numpy as np

import concourse.bacc as bacc
import concourse.tile as tile
from concourse import mybir
from concourse.bass_utils import run_bass_kernel_spmd
from concourse.masks import make_identity

P = 128

# Problem constants (hardcoded per the grading contract).
ALPHA = 0.1
LAMDA = 0.5
LAYER = 1
LN_EPS = 1e-5
LEAKY_SLOPE = 0.01


class Cfg:
    def __init__(self, n_nodes, n_edges, n_cores, rows_per_core, nb, sb, nparts,
                 gather_bufs=4):
        self.N = n_nodes
        self.E = n_edges
        self.NCORES = n_cores
        self.RPC = rows_per_core          # real rows per core
        self.NB = nb                      # 128-row blocks per core (padded)
        self.SB = sb                      # blocks per superstep
        assert nb % sb == 0
        self.NSTEP = nb // sb
        self.NPARTS = nparts              # gather-table splits (int16 idx limit)
        assert n_nodes % nparts == 0
        self.PART = n_nodes // nparts
        assert self.PART <= 32768
        self.C = None                     # groups per (block, part) — from data
        self.gather_bufs = gather_bufs
        self.debug_stage = "full"         # side | hi | resid | noln | full

    @property
    def call_idxs(self):
        return self.SB * self.C * P


FULL_CFG = Cfg(n_nodes=100000, n_edges=1600000, n_cores=8,
               rows_per_core=12500, nb=100, sb=5, nparts=4, gather_bufs=8)


def preprocess(cfg, ego_embeddings, h0, vals, row, col, weight, w_h0, b_h0,
               w_lin, b_lin, gamma, beta_ln):
    """Host-side sharding: bucket/pad edges, build aux tensors per core."""
    ego = ego_embeddings
    N, E, NB, SB, C0 = cfg.N, cfg.E, cfg.NB, cfg.SB, cfg.C
    NPARTS, PART = cfg.NPARTS, cfg.PART
    ego = np.asarray(ego, np.float32)
    h0 = np.asarray(h0, np.float32)
    vals = np.asarray(vals, np.float32)
    row = np.asarray(row)
    col = np.asarray(col)

    core_of = row // cfg.RPC
    np.clip(core_of, 0, cfg.NCORES - 1, out=core_of)

    per_core = []
    cmax = 0
    for k in range(cfg.NCORES):
        m = core_of == k
        r = row[m] - k * cfg.RPC
        c = col[m]
        v = vals[m] * (1.0 - ALPHA)
        blk = r >> 7
        part = c // PART
        cell = blk * NPARTS + part
        counts = np.bincount(cell, minlength=NB * NPARTS)
        cmax = max(cmax, int(counts.max()))
        per_core.append((r, c, v, cell, counts))
    C = max(1, math.ceil(cmax / P))
    if C0 is not None:
        C = max(C, C0)
    cfg.C = C
    call = cfg.call_idxs
    cap = C * P                          # edge slots per (block, part) cell

    wt = np.asarray(weight, np.float32)
    beta = float(np.log(LAMDA / LAYER + 1.0))
    im = (1.0 - beta) + beta * wt                       # [i, o]
    w01t = ALPHA * np.asarray(w_h0, np.float32).T       # [i, o]
    wlint = np.asarray(w_lin, np.float32).T             # [i, o]
    b1 = ALPHA * np.asarray(b_h0, np.float32)
    b2 = np.asarray(b_lin, np.float32)
    gamma = np.asarray(gamma, np.float32)
    beta_ln = np.asarray(beta_ln, np.float32)
    gb_trivial = bool(np.all(gamma == 1.0) and np.all(beta_ln == 0.0))
    wconsts = np.zeros((3 * P + 5, P), np.float32)
    wconsts[0:P] = w01t
    wconsts[P:2 * P] = im
    wconsts[2 * P:3 * P] = wlint
    wconsts[3 * P + 0] = b1
    wconsts[3 * P + 1] = b2
    wconsts[3 * P + 2] = gamma
    wconsts[3 * P + 3] = beta_ln

    in_maps = []
    for k in range(cfg.NCORES):
        r, c, v, cell, counts = per_core[k]
        order = np.argsort(cell, kind="stable")
        r, c, v, cell = r[order], c[order], v[order], cell[order]
        starts = np.zeros(NB * NPARTS, np.int64)
        np.cumsum(counts[:-1], out=starts[1:])
        within = np.arange(len(cell)) - starts[cell]

        blk = cell // NPARTS
        part = cell % NPARTS
        s = blk // SB
        lb = blk % SB
        # flat slot in [NSTEP, NPARTS, call] space
        fpos = (s * NPARTS + part) * call + lb * cap + within

        total = cfg.NSTEP * NPARTS * call
        lcol = np.zeros(total, np.int64)
        val_f = np.zeros(total, np.float32)
        slot_f = np.zeros(total, np.float32)
        lcol[fpos] = c % PART
        val_f[fpos] = v
        slot_f[fpos] = (r & 127).astype(np.float32)

        # gather indices: wrapped int16 layout [s, q, 128, call//16]
        lc = lcol.reshape(cfg.NSTEP, NPARTS, call // 16, 16)
        gidx = np.ascontiguousarray(
            np.broadcast_to(lc.transpose(0, 1, 3, 2)[:, :, None, :, :],
                            (cfg.NSTEP, NPARTS, 8, 16, call // 16))
            .reshape(cfg.NSTEP, NPARTS, P, call // 16)).astype(np.int16)

        # rowval layout [NSTEP, 128, NPARTS, 2, SB*C]; slot j=g*128+p -> [p, g]
        ngc = SB * C
        sv = np.stack([slot_f.reshape(cfg.NSTEP, NPARTS, ngc, P),
                       val_f.reshape(cfg.NSTEP, NPARTS, ngc, P)], axis=2)
        rowval = np.ascontiguousarray(sv.transpose(0, 4, 1, 2, 3)).astype(np.float32)

        base = k * cfg.RPC
        npad = NB * P
        ego_pad = np.zeros((npad, P), np.float32)
        nreal = min(cfg.RPC, cfg.N - base)
        ego_pad[:nreal] = ego[base:base + nreal]
        h0_pad = np.zeros((npad, P), np.float32)
        h0_pad[:nreal] = h0[base:base + nreal]
        ego09 = np.ascontiguousarray(
            (0.9 * ego_pad).reshape(NB, P, P).transpose(1, 0, 2).reshape(P, NB * P))
        h0t = np.ascontiguousarray(h0_pad.T)            # [128, NB*128]

        in_maps.append({
            "ego": ego, "gidx": gidx, "rowval": rowval,
            "ego09": ego09, "h0t": h0t, "wconsts": wconsts,
        })
    return in_maps, gb_trivial


def build_program(cfg, gb_trivial):
    nc = bacc.Bacc("TRN2", target_bir_lowering=False, debug=False)
    f32, i16 = mybir.dt.float32, mybir.dt.int16
    NB, SB, C, NPARTS = cfg.NB, cfg.SB, cfg.C, cfg.NPARTS
    NSTEP, call = cfg.NSTEP, cfg.call_idxs
    ngc = SB * C

    ego = nc.dram_tensor("ego", [cfg.N, P], f32, kind="ExternalInput")
    gidx = nc.dram_tensor("gidx", [NSTEP, NPARTS, P, call // 16], i16,
                          kind="ExternalInput")
    rowval = nc.dram_tensor("rowval", [NSTEP, P, NPARTS, 2, ngc], f32,
                            kind="ExternalInput")
    ego09 = nc.dram_tensor("ego09", [P, NB * P], f32, kind="ExternalInput")
    h0t = nc.dram_tensor("h0t", [P, NB * P], f32, kind="ExternalInput")
    wconsts = nc.dram_tensor("wconsts", [3 * P + 5, P], f32, kind="ExternalInput")
    out = nc.dram_tensor("out", [P, NB * P], f32, kind="ExternalOutput")

    AOP = mybir.AluOpType
    ACT = mybir.ActivationFunctionType

    with tile.TileContext(nc) as tc, ExitStack() as ctx:
        const = ctx.enter_context(tc.tile_pool(name="const", bufs=1))
        gpool = ctx.enter_context(tc.tile_pool(name="gath", bufs=cfg.gather_bufs))
        ipool = ctx.enter_context(tc.tile_pool(name="idx", bufs=4))
        spool = ctx.enter_context(tc.tile_pool(name="step", bufs=2))
        stp = ctx.enter_context(tc.tile_pool(name="st", bufs=8))
        work = ctx.enter_context(tc.tile_pool(name="work", bufs=3))
        small = ctx.enter_context(tc.tile_pool(name="small", bufs=6))
        pside = ctx.enter_context(tc.tile_pool(name="pside", bufs=2, space="PSUM"))
        ppipe = ctx.enter_context(tc.tile_pool(name="ppipe", bufs=4, space="PSUM"))

        w01t_t = const.tile([P, P], f32)
        nc.sync.dma_start(out=w01t_t[:], in_=wconsts[0:P, :])
        im_t = const.tile([P, P], f32)
        nc.sync.dma_start(out=im_t[:], in_=wconsts[P:2 * P, :])
        wlint_t = const.tile([P, P], f32)
        nc.sync.dma_start(out=wlint_t[:], in_=wconsts[2 * P:3 * P, :])
        b1_t = const.tile([1, P], f32)
        nc.sync.dma_start(out=b1_t[:], in_=wconsts[3 * P:3 * P + 1, :])
        b2_t = const.tile([1, P], f32)
        nc.sync.dma_start(out=b2_t[:], in_=wconsts[3 * P + 1:3 * P + 2, :])
        ones_t = const.tile([1, P], f32)
        nc.vector.memset(ones_t[:], 1.0)
        eps_t = const.tile([P, 1], f32)
        nc.vector.memset(eps_t[:], LN_EPS)
        iota_t = const.tile([P, P], f32)
        nc.gpsimd.iota(iota_t[:], [[1, P]], channel_multiplier=0,
                       allow_small_or_imprecise_dtypes=True)
        ident_t = const.tile([P, P], f32)
        make_identity(nc, ident_t[:])
        if not gb_trivial:
            import concourse.bass as bass
            gam_t = const.tile([P, P], f32)
            bet_t = const.tile([P, P], f32)
            grow = wconsts[3 * P + 2:3 * P + 3, :]
            brow = wconsts[3 * P + 3:3 * P + 4, :]
            gb = bass.AP(tensor=grow.tensor, offset=grow.offset,
                         ap=[[0, P]] + grow.ap[1:])
            bb = bass.AP(tensor=brow.tensor, offset=brow.offset,
                         ap=[[0, P]] + brow.ap[1:])
            nc.gpsimd.dma_start(out=gam_t[:], in_=gb)
            nc.gpsimd.dma_start(out=bet_t[:], in_=bb)

        for s in range(NSTEP):
            dsts = []
            for q in range(NPARTS):
                it = ipool.tile([P, call // 16], i16, tag="idx")
                nc.sync.dma_start(out=it[:], in_=gidx[s, q, :, :])
                dst = gpool.tile([P, ngc, P], f32, tag="g")
                nc.gpsimd.dma_gather(dst[:], ego[q * cfg.PART:(q + 1) * cfg.PART, :],
                                     it[:], call, call, P, single_packet=False)
                dsts.append(dst)
            rv_t = spool.tile([P, NPARTS, 2, ngc], f32, tag="rv")
            nc.sync.dma_start(out=rv_t[:], in_=rowval[s, :, :, :, :])
            ego09_t = spool.tile([P, SB * P], f32, tag="e9")
            nc.sync.dma_start(out=ego09_t[:], in_=ego09[:, s * SB * P:(s + 1) * SB * P])
            h0t_t = spool.tile([P, SB * P], f32, tag="h0")
            nc.sync.dma_start(out=h0t_t[:], in_=h0t[:, s * SB * P:(s + 1) * SB * P])
            out_t = spool.tile([P, SB * P], f32, tag="out")

            if cfg.debug_stage == "gather":
                for lb in range(SB):
                    nc.vector.tensor_copy(out=out_t[:, lb * P:(lb + 1) * P],
                                          in_=dsts[lb % NPARTS][:, lb * C, :])
                nc.sync.dma_start(out=out[:, s * SB * P:(s + 1) * SB * P],
                                  in_=out_t[:])
                continue
            for lb in range(SB):
                side = pside.tile([P, P], f32, space="PSUM", tag="side")
                for q in range(NPARTS):
                    for cc in range(C):
                        g = lb * C + cc
                        st = stp.tile([P, P], f32, tag="st")
                        nc.vector.tensor_scalar(
                            out=st[:], in0=iota_t[:],
                            scalar1=rv_t[:, q, 0, g:g + 1],
                            scalar2=rv_t[:, q, 1, g:g + 1],
                            op0=AOP.is_equal, op1=AOP.mult)
                        nc.tensor.matmul(
                            out=side[:], lhsT=st[:], rhs=dsts[q][:, g, :],
                            start=(q == 0 and cc == 0),
                            stop=(q == NPARTS - 1 and cc == C - 1))

                nsl = slice(lb * P, (lb + 1) * P)
                if cfg.debug_stage == "side":
                    nc.vector.tensor_copy(out=out_t[:, nsl], in_=side[:])
                    continue
                hi = work.tile([P, P], f32, tag="hi")
                nc.vector.tensor_add(hi[:], side[:], ego09_t[:, nsl])
                if cfg.debug_stage == "hi":
                    nc.vector.tensor_copy(out=out_t[:, nsl], in_=hi[:])
                    continue

                x_ps = ppipe.tile([P, P], f32, space="PSUM", tag="pp")
                nc.tensor.matmul(out=x_ps[:], lhsT=hi[:], rhs=ident_t[:],
                                 start=True, stop=False)
                nc.tensor.matmul(out=x_ps[:], lhsT=w01t_t[:], rhs=h0t_t[:, nsl],
                                 start=False, stop=False)
                nc.tensor.matmul(out=x_ps[:], lhsT=b1_t[:], rhs=ones_t[:],
                                 start=False, stop=True)
                resid = work.tile([P, P], f32, tag="resid")
                nc.scalar.activation(out=resid[:], in_=x_ps[:], func=ACT.Copy)
                if cfg.debug_stage == "resid":
                    nc.vector.tensor_copy(out=out_t[:, nsl], in_=resid[:])
                    continue

                e_ps = ppipe.tile([P, P], f32, space="PSUM", tag="pp")
                nc.tensor.matmul(out=e_ps[:], lhsT=im_t[:], rhs=resid[:],
                                 start=True, stop=True)
                emb = work.tile([P, P], f32, tag="emb")
                nc.scalar.activation(out=emb[:], in_=e_ps[:], func=ACT.Copy)

                z_ps = ppipe.tile([P, P], f32, space="PSUM", tag="pp")
                nc.tensor.matmul(out=z_ps[:], lhsT=wlint_t[:], rhs=emb[:],
                                 start=True, stop=False)
                nc.tensor.matmul(out=z_ps[:], lhsT=b2_t[:], rhs=ones_t[:],
                                 start=False, stop=True)
                tl = work.tile([P, P], f32, tag="tl")
                nc.vector.tensor_scalar_mul(tl[:], z_ps[:], LEAKY_SLOPE)
                y = work.tile([P, P], f32, tag="y")
                nc.vector.tensor_tensor(out=y[:], in0=z_ps[:], in1=tl[:],
                                        op=AOP.max)

                y_ps = ppipe.tile([P, P], f32, space="PSUM", tag="pp")
                nc.tensor.matmul(out=y_ps[:], lhsT=y[:], rhs=ident_t[:],
                                 start=True, stop=True)

                if cfg.debug_stage == "noln":
                    nc.vector.tensor_copy(out=out_t[:, nsl], in_=y_ps[:])
                    continue
                stats = small.tile([P, 6], f32, tag="bn")
                nc.vector.bn_stats(out=stats[:], in_=y_ps[:])
                mv = small.tile([P, 2], f32, tag="mv")
                nc.vector.bn_aggr(out=mv[:], in_=stats[:])
                sd = small.tile([P, 1], f32, tag="sd")
                nc.scalar.activation(out=sd[:], in_=mv[:, 1:2], func=ACT.Sqrt,
                                     bias=eps_t[:], scale=1.0)
                rstd = small.tile([P, 1], f32, tag="rstd")
                nc.vector.reciprocal(out=rstd[:], in_=sd[:])

                nc.vector.tensor_scalar(
                    out=out_t[:, nsl], in0=y_ps[:],
                    scalar1=mv[:, 0:1], scalar2=rstd[:, 0:1],
                    op0=AOP.subtract, op1=AOP.mult)
                if not gb_trivial:
                    nc.vector.tensor_mul(out_t[:, nsl], out_t[:, nsl], gam_t[:])
                    nc.vector.tensor_add(out_t[:, nsl], out_t[:, nsl], bet_t[:])

            nc.sync.dma_start(out=out[:, s * SB * P:(s + 1) * SB * P], in_=out_t[:])

    nc.compile()
    return nc


def postprocess(cfg, results):
    """Concatenate per-core sb-layout outputs back to [N, 128]."""
    outs = []
    for k in range(cfg.NCORES):
        o = results[k]["out"]                      # [128, NB*128]
        o = o.reshape(P, cfg.NB, P).transpose(1, 0, 2).reshape(cfg.NB * P, P)
        outs.append(o[:cfg.RPC])
    full = np.concatenate(outs, axis=0)[:cfg.N]
    return np.ascontiguousarray(full)


def run(cfg, inputs, trace=False, **kw):
    in_maps, gb_trivial = preprocess(cfg, **inputs)
    nc = build_program(cfg, gb_trivial)
    res = run_bass_kernel_spmd(nc, in_maps, core_ids=list(range(cfg.NCORES)),
                               trace=trace, **kw)
    return postprocess(cfg, res.results), res


def kernel(**inputs) -> np.ndarray:
    out, _ = run(FULL_CFG, inputs)
    return out


# revision 13
# speedup vs baseline: 1.3416x; 1.3416x over previous
"""Trainium2 Bass kernel for nn_Aggregator (GNN message passing + GCNII layer).

Computes, for N=100000 nodes / E=1600000 edges / D=128:
    side = segment_sum(vals * ego[col], row)          # sparse A @ ego
    hi   = ego + side
    res  = 0.9*hi + 0.1*(h0 @ w_h0.T + b_h0)
    emb  = leaky_relu(res @ IM @ w_lin.T + b_lin)     # IM = (1-b) + b*weight
    out  = layernorm(emb) * gamma + beta

Sharding: 8 cores, each owns a contiguous 12500-row slice of the output
nodes (padded to NB*128). The full ego table is replicated per core and
used as the gather source (no collectives). Edges are bucketed on the host
by (dest block of 128 rows, table quarter), padded to 128-edge groups with
a uniform per-cell group count C so all 8 cores run one identical program.

Device inner loop per 128-row block:
  dma_gather pulls each group's 128 neighbor rows onto partitions;
  DVE builds a one-hot selector S[e,n] = (iota==slot[e]) * val[e];
  TensorE accumulates side[n,f] += S.T @ G in PSUM over the block's groups;
  dense epilogue runs feature-major via PE transposes, with biases folded
  into rank-1 accumulating matmuls and LayerNorm via bn_stats/bn_aggr.
"""

import math
from contextlib import ExitStack

import numpy as np

import concourse.bacc as bacc
import concourse.tile as tile
from concourse import mybir
from concourse.bass_utils import run_bass_kernel_spmd
from concourse.masks import make_identity

P = 128

# Problem constants (hardcoded per the grading contract).
ALPHA = 0.1
LAMDA = 0.5
LAYER = 1
LN_EPS = 1e-5
LEAKY_SLOPE = 0.01


class Cfg:
    def __init__(self, n_nodes, n_edges, n_cores, rows_per_core, nb, sb, nparts,
                 gather_bufs=4):
        self.N = n_nodes
        self.E = n_edges
        self.NCORES = n_cores
        self.RPC = rows_per_core          # real rows per core
        self.NB = nb                      # 128-row blocks per core (padded)
        self.SB = sb                      # blocks per superstep
        assert nb % sb == 0
        self.NSTEP = nb // sb
        self.NPARTS = nparts              # gather-table splits (int16 idx limit)
        assert n_nodes % nparts == 0
        self.PART = n_nodes // nparts
        assert self.PART <= 32768
        self.C = None                     # groups per (block, part) — from data
        self.gather_bufs = gather_bufs
        self.debug_stage = "full"         # side | hi | resid | noln | full

    @property
    def call_idxs(self):
        return self.SB * self.C * P


FULL_CFG = Cfg(n_nodes=100000, n_edges=1600000, n_cores=8,
               rows_per_core=12500, nb=100, sb=5, nparts=4, gather_bufs=8)


def preprocess(cfg, ego_embeddings, h0, vals, row, col, weight, w_h0, b_h0,
               w_lin, b_lin, gamma, beta_ln):
    """Host-side sharding: bucket/pad edges, build aux tensors per core."""
    ego = ego_embeddings
    N, E, NB, SB, C0 = cfg.N, cfg.E, cfg.NB, cfg.SB, cfg.C
    NPARTS, PART = cfg.NPARTS, cfg.PART
    ego = np.asarray(ego, np.float32)
    h0 = np.asarray(h0, np.float32)
    vals = np.asarray(vals, np.float32)
    row = np.asarray(row)
    col = np.asarray(col)

    core_of = row // cfg.RPC
    np.clip(core_of, 0, cfg.NCORES - 1, out=core_of)

    per_core = []
    cmax = 0
    for k in range(cfg.NCORES):
        m = core_of == k
        r = row[m] - k * cfg.RPC
        c = col[m]
        v = vals[m] * (1.0 - ALPHA)
        blk = r >> 7
        part = c // PART
        cell = blk * NPARTS + part
        counts = np.bincount(cell, minlength=NB * NPARTS)
        cmax = max(cmax, int(counts.max()))
        per_core.append((r, c, v, cell, counts))
    C = max(1, math.ceil(cmax / P))
    if C0 is not None:
        C = max(C, C0)
    cfg.C = C
    call = cfg.call_idxs
    cap = C * P                          # edge slots per (block, part) cell

    wt = np.asarray(weight, np.float32)
    beta = float(np.log(LAMDA / LAYER + 1.0))
    im = (1.0 - beta) + beta * wt                       # [i, o]
    w01t = ALPHA * np.asarray(w_h0, np.float32).T       # [i, o]
    wlint = np.asarray(w_lin, np.float32).T             # [i, o]
    b1 = ALPHA * np.asarray(b_h0, np.float32)
    b2 = np.asarray(b_lin, np.float32)
    gamma = np.asarray(gamma, np.float32)
    beta_ln = np.asarray(beta_ln, np.float32)
    gb_trivial = bool(np.all(gamma == 1.0) and np.all(beta_ln == 0.0))
    wconsts = np.zeros((3 * P + 5, P), np.float32)
    wconsts[0:P] = w01t
    wconsts[P:2 * P] = im
    wconsts[2 * P:3 * P] = wlint
    wconsts[3 * P + 0] = b1
    wconsts[3 * P + 1] = b2
    wconsts[3 * P + 2] = gamma
    wconsts[3 * P + 3] = beta_ln

    in_maps = []
    for k in range(cfg.NCORES):
        r, c, v, cell, counts = per_core[k]
        order = np.argsort(cell, kind="stable")
        r, c, v, cell = r[order], c[order], v[order], cell[order]
        starts = np.zeros(NB * NPARTS, np.int64)
        np.cumsum(counts[:-1], out=starts[1:])
        within = np.arange(len(cell)) - starts[cell]

        blk = cell // NPARTS
        part = cell % NPARTS
        s = blk // SB
        lb = blk % SB
        # flat slot in [NSTEP, NPARTS, call] space
        fpos = (s * NPARTS + part) * call + lb * cap + within

        total = cfg.NSTEP * NPARTS * call
        lcol = np.zeros(total, np.int64)
        val_f = np.zeros(total, np.float32)
        slot_f = np.zeros(total, np.float32)
        lcol[fpos] = c % PART
        val_f[fpos] = v
        slot_f[fpos] = (r & 127).astype(np.float32)

        # gather indices: wrapped int16 layout [s, q, 128, call//16]
        lc = lcol.reshape(cfg.NSTEP, NPARTS, call // 16, 16)
        gidx = np.ascontiguousarray(
            np.broadcast_to(lc.transpose(0, 1, 3, 2)[:, :, None, :, :],
                            (cfg.NSTEP, NPARTS, 8, 16, call // 16))
            .reshape(cfg.NSTEP, NPARTS, P, call // 16)).astype(np.int16)

        # rowval layout [NSTEP, 128, NPARTS, 2, SB*C]; slot j=g*128+p -> [p, g]
        ngc = SB * C
        sv = np.stack([slot_f.reshape(cfg.NSTEP, NPARTS, ngc, P),
                       val_f.reshape(cfg.NSTEP, NPARTS, ngc, P)], axis=2)
        rowval = np.ascontiguousarray(sv.transpose(0, 4, 1, 2, 3)).astype(np.float32)

        base = k * cfg.RPC
        npad = NB * P
        ego_pad = np.zeros((npad, P), np.float32)
        nreal = min(cfg.RPC, cfg.N - base)
        ego_pad[:nreal] = ego[base:base + nreal]
        h0_pad = np.zeros((npad, P), np.float32)
        h0_pad[:nreal] = h0[base:base + nreal]
        ego09 = np.ascontiguousarray(
            (0.9 * ego_pad).reshape(NB, P, P).transpose(1, 0, 2).reshape(P, NB * P))
        h0t = np.ascontiguousarray(h0_pad.T)            # [128, NB*128]

        in_maps.append({
            "ego": ego, "gidx": gidx, "rowval": rowval,
            "ego09": ego09, "h0t": h0t, "wconsts": wconsts,
        })
    return in_maps, gb_trivial


def build_program(cfg, gb_trivial):
    nc = bacc.Bacc("TRN2", target_bir_lowering=False, debug=False,
                   num_swdge_queues=2)
    f32, i16 = mybir.dt.float32, mybir.dt.int16
    NB, SB, C, NPARTS = cfg.NB, cfg.SB, cfg.C, cfg.NPARTS
    NSTEP, call = cfg.NSTEP, cfg.call_idxs
    ngc = SB * C

    ego = nc.dram_tensor("ego", [cfg.N, P], f32, kind="ExternalInput")
    gidx = nc.dram_tensor("gidx", [NSTEP, NPARTS, P, call // 16], i16,
                          kind="ExternalInput")
    rowval = nc.dram_tensor("rowval", [NSTEP, P, NPARTS, 2, ngc], f32,
                            kind="ExternalInput")
    ego09 = nc.dram_tensor("ego09", [P, NB * P], f32, kind="ExternalInput")
    h0t = nc.dram_tensor("h0t", [P, NB * P], f32, kind="ExternalInput")
    wconsts = nc.dram_tensor("wconsts", [3 * P + 5, P], f32, kind="ExternalInput")
    out = nc.dram_tensor("out", [P, NB * P], f32, kind="ExternalOutput")

    AOP = mybir.AluOpType
    ACT = mybir.ActivationFunctionType

    with tile.TileContext(nc) as tc, ExitStack() as ctx:
        const = ctx.enter_context(tc.tile_pool(name="const", bufs=1))
        gpool = ctx.enter_context(tc.tile_pool(name="gath", bufs=cfg.gather_bufs))
        ipool = ctx.enter_context(tc.tile_pool(name="idx", bufs=4))
        spool = ctx.enter_context(tc.tile_pool(name="step", bufs=2))
        stp = ctx.enter_context(tc.tile_pool(name="st", bufs=8))
        work = ctx.enter_context(tc.tile_pool(name="work", bufs=3))
        small = ctx.enter_context(tc.tile_pool(name="small", bufs=6))
        pside = ctx.enter_context(tc.tile_pool(name="pside", bufs=2, space="PSUM"))
        ppipe = ctx.enter_context(tc.tile_pool(name="ppipe", bufs=4, space="PSUM"))

        w01t_t = const.tile([P, P], f32)
        nc.sync.dma_start(out=w01t_t[:], in_=wconsts[0:P, :])
        im_t = const.tile([P, P], f32)
        nc.sync.dma_start(out=im_t[:], in_=wconsts[P:2 * P, :])
        wlint_t = const.tile([P, P], f32)
        nc.sync.dma_start(out=wlint_t[:], in_=wconsts[2 * P:3 * P, :])
        b1_t = const.tile([1, P], f32)
        nc.sync.dma_start(out=b1_t[:], in_=wconsts[3 * P:3 * P + 1, :])
        b2_t = const.tile([1, P], f32)
        nc.sync.dma_start(out=b2_t[:], in_=wconsts[3 * P + 1:3 * P + 2, :])
        ones_t = const.tile([1, P], f32)
        nc.vector.memset(ones_t[:], 1.0)
        eps_t = const.tile([P, 1], f32)
        nc.vector.memset(eps_t[:], LN_EPS)
        # 129-wide iota/selector tiles: odd free dim keeps the DVE
        # tensor_scalar off 2-port mode, which would contend with GpSimd's
        # SWDGE descriptor generation for the shared SBUF port pair
        # (observed as 20us DVE stalls per dma_gather).
        iota_t = const.tile([P, P + 1], f32)
        nc.gpsimd.iota(iota_t[:], [[1, P + 1]], channel_multiplier=0,
                       allow_small_or_imprecise_dtypes=True)
        ident_t = const.tile([P, P], f32)
        make_identity(nc, ident_t[:])
        if not gb_trivial:
            import concourse.bass as bass
            gam_t = const.tile([P, P], f32)
            bet_t = const.tile([P, P], f32)
            grow = wconsts[3 * P + 2:3 * P + 3, :]
            brow = wconsts[3 * P + 3:3 * P + 4, :]
            gb = bass.AP(tensor=grow.tensor, offset=grow.offset,
                         ap=[[0, P]] + grow.ap[1:])
            bb = bass.AP(tensor=brow.tensor, offset=brow.offset,
                         ap=[[0, P]] + brow.ap[1:])
            nc.gpsimd.dma_start(out=gam_t[:], in_=gb)
            nc.gpsimd.dma_start(out=bet_t[:], in_=bb)

        for s in range(NSTEP):
            dsts = []
            for q in range(NPARTS):
                it = ipool.tile([P, call // 16], i16, tag="idx")
                nc.sync.dma_start(out=it[:], in_=gidx[s, q, :, :])
                dst = gpool.tile([P, ngc, P], f32, tag="g")
                nc.gpsimd.dma_gather(dst[:], ego[q * cfg.PART:(q + 1) * cfg.PART, :],
                                     it[:], call, call, P, single_packet=False,
                                     queue_num=q % 2)
                dsts.append(dst)
            rv_t = spool.tile([P, NPARTS, 2, ngc], f32, tag="rv")
            nc.sync.dma_start(out=rv_t[:], in_=rowval[s, :, :, :, :])
            ego09_t = spool.tile([P, SB * P], f32, tag="e9")
            nc.sync.dma_start(out=ego09_t[:], in_=ego09[:, s * SB * P:(s + 1) * SB * P])
            h0t_t = spool.tile([P, SB * P], f32, tag="h0")
            nc.sync.dma_start(out=h0t_t[:], in_=h0t[:, s * SB * P:(s + 1) * SB * P])
            out_t = spool.tile([P, SB * P], f32, tag="out")

            if cfg.debug_stage == "gather":
                for lb in range(SB):
                    nc.vector.tensor_copy(out=out_t[:, lb * P:(lb + 1) * P],
                                          in_=dsts[lb % NPARTS][:, lb * C, :])
                nc.sync.dma_start(out=out[:, s * SB * P:(s + 1) * SB * P],
                                  in_=out_t[:])
                continue
            for lb in range(SB):
                side = pside.tile([P, P], f32, space="PSUM", tag="side")
                for q in range(NPARTS):
                    for cc in range(C):
                        g = lb * C + cc
                        st = stp.tile([P, P + 1], f32, tag="st")
                        nc.vector.tensor_scalar(
                            out=st[:], in0=iota_t[:],
                            scalar1=rv_t[:, q, 0, g:g + 1],
                            scalar2=rv_t[:, q, 1, g:g + 1],
                            op0=AOP.is_equal, op1=AOP.mult)
                        nc.tensor.matmul(
                            out=side[:], lhsT=st[:, :P], rhs=dsts[q][:, g, :],
                            start=(q == 0 and cc == 0),
                            stop=(q == NPARTS - 1 and cc == C - 1))

                nsl = slice(lb * P, (lb + 1) * P)
                if cfg.debug_stage == "side":
                    nc.vector.tensor_copy(out=out_t[:, nsl], in_=side[:])
                    continue
                hi = work.tile([P, P], f32, tag="hi")
                nc.vector.tensor_add(hi[:], side[:], ego09_t[:, nsl])
                if cfg.debug_stage == "hi":
                    nc.vector.tensor_copy(out=out_t[:, nsl], in_=hi[:])
                    continue

                x_ps = ppipe.tile([P, P], f32, space="PSUM", tag="pp")
                nc.tensor.matmul(out=x_ps[:], lhsT=hi[:], rhs=ident_t[:],
                                 start=True, stop=False)
                nc.tensor.matmul(out=x_ps[:], lhsT=w01t_t[:], rhs=h0t_t[:, nsl],
                                 start=False, stop=False)
                nc.tensor.matmul(out=x_ps[:], lhsT=b1_t[:], rhs=ones_t[:],
                                 start=False, stop=True)
                resid = work.tile([P, P], f32, tag="resid")
                nc.scalar.activation(out=resid[:], in_=x_ps[:], func=ACT.Copy)
                if cfg.debug_stage == "resid":
                    nc.vector.tensor_copy(out=out_t[:, nsl], in_=resid[:])
                    continue

                e_ps = ppipe.tile([P, P], f32, space="PSUM", tag="pp")
                nc.tensor.matmul(out=e_ps[:], lhsT=im_t[:], rhs=resid[:],
                                 start=True, stop=True)
                emb = work.tile([P, P], f32, tag="emb")
                nc.scalar.activation(out=emb[:], in_=e_ps[:], func=ACT.Copy)

                z_ps = ppipe.tile([P, P], f32, space="PSUM", tag="pp")
                nc.tensor.matmul(out=z_ps[:], lhsT=wlint_t[:], rhs=emb[:],
                                 start=True, stop=False)
                nc.tensor.matmul(out=z_ps[:], lhsT=b2_t[:], rhs=ones_t[:],
                                 start=False, stop=True)
                tl = work.tile([P, P], f32, tag="tl")
                nc.vector.tensor_scalar_mul(tl[:], z_ps[:], LEAKY_SLOPE)
                y = work.tile([P, P], f32, tag="y")
                nc.vector.tensor_tensor(out=y[:], in0=z_ps[:], in1=tl[:],
                                        op=AOP.max)

                y_ps = ppipe.tile([P, P], f32, space="PSUM", tag="pp")
                nc.tensor.matmul(out=y_ps[:], lhsT=y[:], rhs=ident_t[:],
                                 start=True, stop=True)

                if cfg.debug_stage == "noln":
                    nc.vector.tensor_copy(out=out_t[:, nsl], in_=y_ps[:])
                    continue
                stats = small.tile([P, 6], f32, tag="bn")
                nc.vector.bn_stats(out=stats[:], in_=y_ps[:])
                mv = small.tile([P, 2], f32, tag="mv")
                nc.vector.bn_aggr(out=mv[:], in_=stats[:])
                sd = small.tile([P, 1], f32, tag="sd")
                nc.scalar.activation(out=sd[:], in_=mv[:, 1:2], func=ACT.Sqrt,
                                     bias=eps_t[:], scale=1.0)
                rstd = small.tile([P, 1], f32, tag="rstd")
                nc.vector.reciprocal(out=rstd[:], in_=sd[:])

                nc.vector.tensor_scalar(
                    out=out_t[:, nsl], in0=y_ps[:],
                    scalar1=mv[:, 0:1], scalar2=rstd[:, 0:1],
                    op0=AOP.subtract, op1=AOP.mult)
                if not gb_trivial:
                    nc.vector.tensor_mul(out_t[:, nsl], out_t[:, nsl], gam_t[:])
                    nc.vector.tensor_add(out_t[:, nsl], out_t[:, nsl], bet_t[:])

            nc.sync.dma_start(out=out[:, s * SB * P:(s + 1) * SB * P], in_=out_t[:])

    nc.compile()
    return nc


def postprocess(cfg, results):
    """Concatenate per-core sb-layout outputs back to [N, 128]."""
    outs = []
    for k in range(cfg.NCORES):
        o = results[k]["out"]                      # [128, NB*128]
        o = o.reshape(P, cfg.NB, P).transpose(1, 0, 2).reshape(cfg.NB * P, P)
        outs.append(o[:cfg.RPC])
    full = np.concatenate(outs, axis=0)[:cfg.N]
    return np.ascontiguousarray(full)


def run(cfg, inputs, trace=False, **kw):
    in_maps, gb_trivial = preprocess(cfg, **inputs)
    nc = build_program(cfg, gb_trivial)
    res = run_bass_kernel_spmd(nc, in_maps, core_ids=list(range(cfg.NCORES)),
                               trace=trace, **kw)
    return postprocess(cfg, res.results), res


def kernel(**inputs) -> np.ndarray:
    out, _ = run(FULL_CFG, inputs)
    return out


# revision 27
# speedup vs baseline: 1.5149x; 1.1292x over previous
"""Trainium2 Bass kernel for nn_Aggregator (GNN message passing + GCNII layer).

Computes, for N=100000 nodes / E=1600000 edges / D=128:
    side = segment_sum(vals * ego[col], row)          # sparse A @ ego
    hi   = ego + side
    res  = 0.9*hi + 0.1*(h0 @ w_h0.T + b_h0)
    emb  = leaky_relu(res @ IM @ w_lin.T + b_lin)     # IM = (1-b) + b*weight
    out  = layernorm(emb) * gamma + beta

Sharding: 8 cores, each owns a contiguous 12500-row slice of the output
nodes (padded to NB*128). The full ego table is replicated per core and
used as the gather source (no collectives). Edges are bucketed on the host
by (dest block of 128 rows, table quarter), padded to 128-edge groups with
a uniform per-cell group count C so all 8 cores run one identical program.

Device inner loop per 128-row block:
  dma_gather pulls each group's 128 neighbor rows onto partitions;
  DVE builds a one-hot selector S[e,n] = (iota==slot[e]) * val[e];
  TensorE accumulates side[n,f] += S.T @ G in PSUM over the block's groups;
  dense epilogue runs feature-major via PE transposes, with biases folded
  into rank-1 accumulating matmuls and LayerNorm via bn_stats/bn_aggr.
"""

import math
from contextlib import ExitStack

import numpy as np

import concourse.bacc as bacc
import concourse.tile as tile
from concourse import mybir
from concourse.bass_utils import run_bass_kernel_spmd
from concourse.masks import make_identity

P = 128

# Problem constants (hardcoded per the grading contract).
ALPHA = 0.1
LAMDA = 0.5
LAYER = 1
LN_EPS = 1e-5
LEAKY_SLOPE = 0.01


class Cfg:
    def __init__(self, n_nodes, n_edges, n_cores, rows_per_core, nb, sb, nparts,
                 gather_bufs=4):
        self.N = n_nodes
        self.E = n_edges
        self.NCORES = n_cores
        self.RPC = rows_per_core          # real rows per core
        self.NB = nb                      # 128-row blocks per core (padded)
        self.SB = sb                      # blocks per superstep
        assert nb % sb == 0
        self.NSTEP = nb // sb
        self.NPARTS = nparts              # gather-table splits (int16 idx limit)
        assert n_nodes % nparts == 0
        self.PART = n_nodes // nparts
        assert self.PART <= 32768
        self.C = None                     # groups per (block, part) — from data
        self.gather_bufs = gather_bufs
        self.debug_stage = "full"         # side | hi | resid | noln | full

    @property
    def call_idxs(self):
        return self.SB * self.C * P


FULL_CFG = Cfg(n_nodes=100000, n_edges=1600000, n_cores=8,
               rows_per_core=12500, nb=100, sb=5, nparts=4, gather_bufs=6)


def preprocess(cfg, ego_embeddings, h0, vals, row, col, weight, w_h0, b_h0,
               w_lin, b_lin, gamma, beta_ln):
    """Host-side sharding: bucket/pad edges, build aux tensors per core."""
    ego = ego_embeddings
    N, E, NB, SB, C0 = cfg.N, cfg.E, cfg.NB, cfg.SB, cfg.C
    NPARTS, PART = cfg.NPARTS, cfg.PART
    ego = np.asarray(ego, np.float32)
    h0 = np.asarray(h0, np.float32)
    vals = np.asarray(vals, np.float32)
    row = np.asarray(row)
    col = np.asarray(col)

    core_of = row // cfg.RPC
    np.clip(core_of, 0, cfg.NCORES - 1, out=core_of)

    per_core = []
    cmax = 0
    for k in range(cfg.NCORES):
        m = core_of == k
        r = row[m] - k * cfg.RPC
        c = col[m]
        v = vals[m] * (1.0 - ALPHA)
        blk = r >> 7
        part = c // PART
        cell = blk * NPARTS + part
        counts = np.bincount(cell, minlength=NB * NPARTS)
        cmax = max(cmax, int(counts.max()))
        per_core.append((r, c, v, cell, counts))
    C = max(1, math.ceil(cmax / P))
    if C0 is not None:
        C = max(C, C0)
    cfg.C = C
    call = cfg.call_idxs
    cap = C * P                          # edge slots per (block, part) cell

    wt = np.asarray(weight, np.float32)
    beta = float(np.log(LAMDA / LAYER + 1.0))
    im = (1.0 - beta) + beta * wt                       # [i, o]
    w01t = ALPHA * np.asarray(w_h0, np.float32).T       # [i, o]
    wlint = np.asarray(w_lin, np.float32).T             # [i, o]
    b1 = ALPHA * np.asarray(b_h0, np.float32)
    b2 = np.asarray(b_lin, np.float32)
    gamma = np.asarray(gamma, np.float32)
    beta_ln = np.asarray(beta_ln, np.float32)
    gb_trivial = bool(np.all(gamma == 1.0) and np.all(beta_ln == 0.0))
    wconsts = np.zeros((3 * P + 5, P), np.float32)
    wconsts[0:P] = w01t
    wconsts[P:2 * P] = im
    wconsts[2 * P:3 * P] = wlint
    wconsts[3 * P + 0] = b1
    wconsts[3 * P + 1] = b2
    wconsts[3 * P + 2] = gamma
    wconsts[3 * P + 3] = beta_ln

    in_maps = []
    for k in range(cfg.NCORES):
        r, c, v, cell, counts = per_core[k]
        order = np.argsort(cell, kind="stable")
        r, c, v, cell = r[order], c[order], v[order], cell[order]
        starts = np.zeros(NB * NPARTS, np.int64)
        np.cumsum(counts[:-1], out=starts[1:])
        within = np.arange(len(cell)) - starts[cell]

        blk = cell // NPARTS
        part = cell % NPARTS
        s = blk // SB
        lb = blk % SB
        # flat slot in [NSTEP, NPARTS, call] space
        fpos = (s * NPARTS + part) * call + lb * cap + within

        total = cfg.NSTEP * NPARTS * call
        lcol = np.zeros(total, np.int64)
        val_f = np.zeros(total, np.float32)
        slot_f = np.zeros(total, np.float32)
        lcol[fpos] = c % PART
        val_f[fpos] = v
        slot_f[fpos] = (r & 127).astype(np.float32)

        # gather indices: wrapped int16 layout [s, q, 128, call//16]
        lc = lcol.reshape(cfg.NSTEP, NPARTS, call // 16, 16)
        gidx = np.ascontiguousarray(
            np.broadcast_to(lc.transpose(0, 1, 3, 2)[:, :, None, :, :],
                            (cfg.NSTEP, NPARTS, 8, 16, call // 16))
            .reshape(cfg.NSTEP, NPARTS, P, call // 16)).astype(np.int16)

        # host-built dense selectors: S[p, n] = val * (slot == n), laid out
        # [NSTEP, 128p, (q, g, n) flattened] so one DMA loads a superstep.
        # (Building these on DVE serializes against GpSimd's SWDGE descriptor
        # generation — the shared SBUF port pair — so ship them pre-expanded.)
        ngc = SB * C
        total_slots = cfg.NSTEP * NPARTS * call
        sflat = np.zeros((total_slots, P), np.float32)
        sflat[np.arange(total_slots), slot_f.astype(np.int64)] = val_f
        sel = np.ascontiguousarray(
            sflat.reshape(cfg.NSTEP, NPARTS, ngc, P, P).transpose(0, 3, 1, 2, 4)
            .reshape(cfg.NSTEP, P, NPARTS * ngc * P))

        base = k * cfg.RPC
        npad = NB * P
        ego_pad = np.zeros((npad, P), np.float32)
        nreal = min(cfg.RPC, cfg.N - base)
        ego_pad[:nreal] = ego[base:base + nreal]
        h0_pad = np.zeros((npad, P), np.float32)
        h0_pad[:nreal] = h0[base:base + nreal]
        ego09 = np.ascontiguousarray(
            (0.9 * ego_pad).reshape(NB, P, P).transpose(1, 0, 2).reshape(P, NB * P))
        h0t = np.ascontiguousarray(h0_pad.T)            # [128, NB*128]

        in_maps.append({
            "ego": ego, "gidx": gidx, "sel": sel,
            "ego09": ego09, "h0t": h0t, "wconsts": wconsts,
        })
    return in_maps, gb_trivial


def build_program(cfg, gb_trivial):
    nc = bacc.Bacc("TRN2", target_bir_lowering=False, debug=False,
                   num_swdge_queues=2)
    f32, i16 = mybir.dt.float32, mybir.dt.int16
    NB, SB, C, NPARTS = cfg.NB, cfg.SB, cfg.C, cfg.NPARTS
    NSTEP, call = cfg.NSTEP, cfg.call_idxs
    ngc = SB * C

    ego = nc.dram_tensor("ego", [cfg.N, P], f32, kind="ExternalInput")
    gidx = nc.dram_tensor("gidx", [NSTEP, NPARTS, P, call // 16], i16,
                          kind="ExternalInput")
    sel = nc.dram_tensor("sel", [NSTEP, P, NPARTS * ngc * P], f32,
                         kind="ExternalInput")
    ego09 = nc.dram_tensor("ego09", [P, NB * P], f32, kind="ExternalInput")
    h0t = nc.dram_tensor("h0t", [P, NB * P], f32, kind="ExternalInput")
    wconsts = nc.dram_tensor("wconsts", [3 * P + 5, P], f32, kind="ExternalInput")
    out = nc.dram_tensor("out", [P, NB * P], f32, kind="ExternalOutput")

    AOP = mybir.AluOpType
    ACT = mybir.ActivationFunctionType

    with tile.TileContext(nc) as tc, ExitStack() as ctx:
        const = ctx.enter_context(tc.tile_pool(name="const", bufs=1))
        gpool = ctx.enter_context(tc.tile_pool(name="gath", bufs=cfg.gather_bufs))
        ipool = ctx.enter_context(tc.tile_pool(name="idx", bufs=4))
        spool = ctx.enter_context(tc.tile_pool(name="step", bufs=2))
        selp = ctx.enter_context(tc.tile_pool(name="selp", bufs=6))
        work = ctx.enter_context(tc.tile_pool(name="work", bufs=3))
        small = ctx.enter_context(tc.tile_pool(name="small", bufs=6))
        pside = ctx.enter_context(tc.tile_pool(name="pside", bufs=2, space="PSUM"))
        ppipe = ctx.enter_context(tc.tile_pool(name="ppipe", bufs=4, space="PSUM"))

        w01t_t = const.tile([P, P], f32)
        nc.sync.dma_start(out=w01t_t[:], in_=wconsts[0:P, :])
        im_t = const.tile([P, P], f32)
        nc.sync.dma_start(out=im_t[:], in_=wconsts[P:2 * P, :])
        wlint_t = const.tile([P, P], f32)
        nc.sync.dma_start(out=wlint_t[:], in_=wconsts[2 * P:3 * P, :])
        b1_t = const.tile([1, P], f32)
        nc.sync.dma_start(out=b1_t[:], in_=wconsts[3 * P:3 * P + 1, :])
        b2_t = const.tile([1, P], f32)
        nc.sync.dma_start(out=b2_t[:], in_=wconsts[3 * P + 1:3 * P + 2, :])
        ones_t = const.tile([1, P], f32)
        nc.vector.memset(ones_t[:], 1.0)
        eps_t = const.tile([P, 1], f32)
        nc.vector.memset(eps_t[:], LN_EPS)
        ident_t = const.tile([P, P], f32)
        make_identity(nc, ident_t[:])
        if not gb_trivial:
            # broadcast gamma/beta along partitions via a K=1 outer-product
            # matmul (keeps Pool free of non-gather DMAs so the SWDGE
            # queue<->sem-lane pairing stays consistent).
            grow = const.tile([1, P], f32)
            nc.sync.dma_start(out=grow[:], in_=wconsts[3 * P + 2:3 * P + 3, :])
            brow = const.tile([1, P], f32)
            nc.sync.dma_start(out=brow[:], in_=wconsts[3 * P + 3:3 * P + 4, :])
            ones1 = const.tile([1, P], f32)
            nc.vector.memset(ones1[:], 1.0)
            gb_ps = pside.tile([P, 2 * P], f32, space="PSUM", tag="gb")
            nc.tensor.matmul(out=gb_ps[:, :P], lhsT=ones1[:], rhs=grow[:],
                             start=True, stop=True)
            nc.tensor.matmul(out=gb_ps[:, P:], lhsT=ones1[:], rhs=brow[:],
                             start=True, stop=True)
            gam_t = const.tile([P, P], f32)
            nc.scalar.activation(out=gam_t[:], in_=gb_ps[:, :P], func=ACT.Copy)
            bet_t = const.tile([P, P], f32)
            nc.scalar.activation(out=bet_t[:], in_=gb_ps[:, P:], func=ACT.Copy)

        for s in range(NSTEP):
            dsts = []
            for q in range(NPARTS):
                it = ipool.tile([P, call // 16], i16, tag="idx")
                nc.sync.dma_start(out=it[:], in_=gidx[s, q, :, :])
                dst = gpool.tile([P, ngc, P], f32, tag="g")
                nc.gpsimd.dma_gather(dst[:], ego[q * cfg.PART:(q + 1) * cfg.PART, :],
                                     it[:], call, call, P, single_packet=False,
                                     queue_num=q % 2)
                dsts.append(dst)
            sel_ts = []
            for q in range(NPARTS):
                sq = selp.tile([P, ngc, P], f32, tag="sel")
                nc.sync.dma_start(out=sq[:],
                                  in_=sel[s, :, q * ngc * P:(q + 1) * ngc * P])
                sel_ts.append(sq)
            ego09_t = spool.tile([P, SB * P], f32, tag="e9")
            nc.sync.dma_start(out=ego09_t[:], in_=ego09[:, s * SB * P:(s + 1) * SB * P])
            h0t_t = spool.tile([P, SB * P], f32, tag="h0")
            nc.sync.dma_start(out=h0t_t[:], in_=h0t[:, s * SB * P:(s + 1) * SB * P])
            out_t = spool.tile([P, SB * P], f32, tag="out")

            if cfg.debug_stage == "gather":
                for lb in range(SB):
                    nc.vector.tensor_copy(out=out_t[:, lb * P:(lb + 1) * P],
                                          in_=dsts[lb % NPARTS][:, lb * C, :])
                nc.sync.dma_start(out=out[:, s * SB * P:(s + 1) * SB * P],
                                  in_=out_t[:])
                continue
            for lb in range(SB):
                side = pside.tile([P, P], f32, space="PSUM", tag="side")
                for q in range(NPARTS):
                    for cc in range(C):
                        g = lb * C + cc
                        nc.tensor.matmul(
                            out=side[:], lhsT=sel_ts[q][:, g, :],
                            rhs=dsts[q][:, g, :],
                            start=(q == 0 and cc == 0),
                            stop=(q == NPARTS - 1 and cc == C - 1))

                nsl = slice(lb * P, (lb + 1) * P)
                if cfg.debug_stage == "side":
                    nc.vector.tensor_copy(out=out_t[:, nsl], in_=side[:])
                    continue
                hi = work.tile([P, P], f32, tag="hi")
                nc.vector.tensor_add(hi[:], side[:], ego09_t[:, nsl])
                if cfg.debug_stage == "hi":
                    nc.vector.tensor_copy(out=out_t[:, nsl], in_=hi[:])
                    continue

                x_ps = ppipe.tile([P, P], f32, space="PSUM", tag="pp")
                nc.tensor.matmul(out=x_ps[:], lhsT=hi[:], rhs=ident_t[:],
                                 start=True, stop=False)
                nc.tensor.matmul(out=x_ps[:], lhsT=w01t_t[:], rhs=h0t_t[:, nsl],
                                 start=False, stop=False)
                nc.tensor.matmul(out=x_ps[:], lhsT=b1_t[:], rhs=ones_t[:],
                                 start=False, stop=True)
                resid = work.tile([P, P], f32, tag="resid")
                nc.scalar.activation(out=resid[:], in_=x_ps[:], func=ACT.Copy)
                if cfg.debug_stage == "resid":
                    nc.vector.tensor_copy(out=out_t[:, nsl], in_=resid[:])
                    continue

                e_ps = ppipe.tile([P, P], f32, space="PSUM", tag="pp")
                nc.tensor.matmul(out=e_ps[:], lhsT=im_t[:], rhs=resid[:],
                                 start=True, stop=True)
                emb = work.tile([P, P], f32, tag="emb")
                nc.scalar.activation(out=emb[:], in_=e_ps[:], func=ACT.Copy)

                z_ps = ppipe.tile([P, P], f32, space="PSUM", tag="pp")
                nc.tensor.matmul(out=z_ps[:], lhsT=wlint_t[:], rhs=emb[:],
                                 start=True, stop=False)
                nc.tensor.matmul(out=z_ps[:], lhsT=b2_t[:], rhs=ones_t[:],
                                 start=False, stop=True)
                tl = work.tile([P, P], f32, tag="tl")
                nc.vector.tensor_scalar_mul(tl[:], z_ps[:], LEAKY_SLOPE)
                y = work.tile([P, P], f32, tag="y")
                nc.vector.tensor_tensor(out=y[:], in0=z_ps[:], in1=tl[:],
                                        op=AOP.max)

                y_ps = ppipe.tile([P, P], f32, space="PSUM", tag="pp")
                nc.tensor.matmul(out=y_ps[:], lhsT=y[:], rhs=ident_t[:],
                                 start=True, stop=True)

                if cfg.debug_stage == "noln":
                    nc.vector.tensor_copy(out=out_t[:, nsl], in_=y_ps[:])
                    continue
                stats = small.tile([P, 6], f32, tag="bn")
                nc.vector.bn_stats(out=stats[:], in_=y_ps[:])
                mv = small.tile([P, 2], f32, tag="mv")
                nc.vector.bn_aggr(out=mv[:], in_=stats[:])
                sd = small.tile([P, 1], f32, tag="sd")
                nc.scalar.activation(out=sd[:], in_=mv[:, 1:2], func=ACT.Sqrt,
                                     bias=eps_t[:], scale=1.0)
                rstd = small.tile([P, 1], f32, tag="rstd")
                nc.vector.reciprocal(out=rstd[:], in_=sd[:])

                nc.vector.tensor_scalar(
                    out=out_t[:, nsl], in0=y_ps[:],
                    scalar1=mv[:, 0:1], scalar2=rstd[:, 0:1],
                    op0=AOP.subtract, op1=AOP.mult)
                if not gb_trivial:
                    nc.vector.tensor_mul(out_t[:, nsl], out_t[:, nsl], gam_t[:])
                    nc.vector.tensor_add(out_t[:, nsl], out_t[:, nsl], bet_t[:])

            nc.sync.dma_start(out=out[:, s * SB * P:(s + 1) * SB * P], in_=out_t[:])

    nc.compile()
    return nc


def postprocess(cfg, results):
    """Concatenate per-core sb-layout outputs back to [N, 128]."""
    outs = []
    for k in range(cfg.NCORES):
        o = results[k]["out"]                      # [128, NB*128]
        o = o.reshape(P, cfg.NB, P).transpose(1, 0, 2).reshape(cfg.NB * P, P)
        outs.append(o[:cfg.RPC])
    full = np.concatenate(outs, axis=0)[:cfg.N]
    return np.ascontiguousarray(full)


def run(cfg, inputs, trace=False, **kw):
    in_maps, gb_trivial = preprocess(cfg, **inputs)
    nc = build_program(cfg, gb_trivial)
    res = run_bass_kernel_spmd(nc, in_maps, core_ids=list(range(cfg.NCORES)),
                               trace=trace, **kw)
    return postprocess(cfg, res.results), res


def kernel(**inputs) -> np.ndarray:
    out, _ = run(FULL_CFG, inputs)
    return out
